# revision 27
# baseline (speedup 1.0000x reference)
"""Multi-head attention (B=2, S=2048, D=768, H=12) on 8 TRN2 NeuronCores.

Sharding: batch x head-group. Core c handles batch c//4 and heads
[3*(c%4), 3*(c%4)+3). Each core computes Q/K/V projections for its 3
heads, scores + softmax (written to HBM - the dominant 402MB output),
context and a partial output projection. Host sums the 4 partial
outputs per batch and adds the output bias.

Device pipeline per core (all matmuls fp32r on the PE except PV/out
which run bf16):
  - load q^T,k^T,v^T (pre-transposed on host) via gpsimd cast-DMA (fp32->fp32r)
  - project Q^T,K^T (fp32r, [dh,s] layout) and V^T -> PE-transpose -> V [s_k,dh]
  - per head: scores twice: natural orientation (softmax rows on
    partitions -> exp with free rowsum accumulation -> normalize -> DMA out)
    and transposed orientation (exp -> E^T bf16 -> PV matmul)
  - ctx normalized by 1/rowsum during PV psum evacuation, PE-transposed,
    projected through Wo^T -> partial out^T -> DMA
"""

import numpy as np

B, S, D, H, DH = 2, 2048, 768, 12, 64
HPC = 3          # heads per core
NLOC = HPC * DH  # 192 local head dims
N_CORES = 8

_CACHE = {}
LAST_RESULT = None  # BassKernelResults of the most recent run (for test.py)


def _build_nc():
    import concourse.bacc as bacc
    import concourse.mybir as mybir
    import concourse.tile as tile
    from concourse.masks import make_identity

    F32 = mybir.dt.float32
    F32R = mybir.dt.float32r
    BF16 = mybir.dt.bfloat16
    EXP = mybir.ActivationFunctionType.Exp
    ADD = mybir.AluOpType.add

    nc = bacc.Bacc("TRN2", target_bir_lowering=False)

    qt_d = nc.declare_dram_parameter("qt", [D, S], F32, isOutput=False)
    kt_d = nc.declare_dram_parameter("kt", [D, S], F32, isOutput=False)
    vt_d = nc.declare_dram_parameter("vt", [D, S], BF16, isOutput=False)
    wqt_d = nc.declare_dram_parameter("wqt", [D, NLOC], F32, isOutput=False)
    wkt_d = nc.declare_dram_parameter("wkt", [D, NLOC], F32, isOutput=False)
    wvt_d = nc.declare_dram_parameter("wvt", [D, NLOC], BF16, isOutput=False)
    wot_d = nc.declare_dram_parameter("wot", [NLOC, D], F32, isOutput=False)
    biasA_d = nc.declare_dram_parameter("biasA", [128, 3], F32, isOutput=False)
    biasB_d = nc.declare_dram_parameter("biasB", [64, 3], F32, isOutput=False)
    biasV_d = nc.declare_dram_parameter("biasV", [64, 2], F32, isOutput=False)
    scores_d = nc.declare_dram_parameter("scores", [HPC, S, S], F32, isOutput=True)
    outt_d = nc.declare_dram_parameter("outt", [D, S], F32, isOutput=True)

    with tile.TileContext(nc) as tc:
        with (
            tc.tile_pool(name="consts", bufs=1) as consts,
            tc.tile_pool(name="stgp", bufs=3) as stgp,
            tc.tile_pool(name="bigp", bufs=1) as bigp,
            tc.tile_pool(name="etp", bufs=3) as etp,
            tc.tile_pool(name="pp", bufs=2) as pp,
            tc.tile_pool(name="op", bufs=2) as op,
            tc.tile_pool(name="zp", bufs=3) as zp,
            tc.tile_pool(name="psum", bufs=2, space="PSUM") as psum,
        ):
            # ---- constants ----
            identB = consts.tile([128, 128], BF16)
            make_identity(nc, identB)
            wq_t = consts.tile([128, 6, NLOC], F32R)
            wk_t = consts.tile([128, 6, NLOC], F32R)
            wv_t = consts.tile([128, 6, NLOC], BF16)
            wot_t = consts.tile([64, 3, D], BF16)

            def load_w(w_t, w_d):
                nc.gpsimd.dma_start(
                    out=w_t, in_=w_d[:].rearrange("(c p) n -> p c n", p=128))
            biasA = consts.tile([128, 3], F32)
            biasB = consts.tile([128, 3], F32)
            nc.sync.dma_start(biasA[:], biasA_d[:])
            nc.sync.dma_start(biasB[0:64, :], biasB_d[:])
            biasV = consts.tile([64, 2], F32)
            nc.sync.dma_start(biasV[:], biasV_d[:])

            # ---- persistent activations ----
            QTa = bigp.tile([128, S], F32R)   # head0 rows 0:64, head1 rows 64:128
            QT2 = bigp.tile([128, S], F32R)   # head2 rows 64:128
            KTa = bigp.tile([128, S], F32R)
            KT2 = bigp.tile([128, S], F32R)
            VT3 = bigp.tile([64, 3, S], BF16)  # V^T per head, rows 0:64
            V_sb = bigp.tile([128, 16, NLOC], BF16)   # V natural [s_k, dh]
            ctxn = bigp.tile([128, 16, NLOC], BF16)   # ctx natural, normalized
            ctxT = bigp.tile([64, 3, S], BF16)        # ctx^T per head

            # ---- phase P: projections ----
            # out^T[dh, s] accumulated over 6 D-chunks streamed in 2 halves.
            # (lhsT weight col slice, psum row slice, sbuf dest fn, bias AP)
            projs = [
                (qt_d, wq_t,
                 [(slice(0, 128), lambda sl: QTa[:, sl], biasA[:, 0:1]),
                  (slice(128, 192), lambda sl: QT2[0:64, sl], biasB[0:64, 0:1])]),
                (kt_d, wk_t,
                 [(slice(0, 128), lambda sl: KTa[:, sl], biasA[:, 1:2]),
                  (slice(128, 192), lambda sl: KT2[0:64, sl], biasB[0:64, 1:2])]),
                (vt_d, wv_t,
                 [(slice(0, 64), lambda sl: VT3[0:64, 0, sl], biasV[:, 0:1]),
                  (slice(64, 128), lambda sl: VT3[0:64, 1, sl], biasV[:, 1:2]),
                  (slice(128, 192), lambda sl: VT3[0:64, 2, sl], biasB[0:64, 2:3])]),
            ]
            stg_tiles = {}

            def emit_proj_load(ti, g):
                # load one 512-column slice of x^T (all 6 D-chunks)
                x_d, _, _ = projs[ti]
                sl = slice(g * 512, (g + 1) * 512)
                stg = stgp.tile([128, 6, 512], BF16 if ti == 2 else F32R, tag="stg")
                nc.gpsimd.dma_start(
                    out=stg,
                    in_=x_d[:, sl].rearrange("(c p) s -> p c s", p=128),
                )
                stg_tiles[(ti, g)] = stg

            def emit_proj_part(ti, g, pi):
                # project one output-row group of one slice (full contraction)
                _, w_t, parts = projs[ti]
                sl = slice(g * 512, (g + 1) * 512)
                stg = stg_tiles[(ti, g)]
                wsl, dst_fn, bias_ap = parts[pi]
                m = wsl.stop - wsl.start
                psA = psum.tile([128, 512], F32, tag="m5", bufs=2)
                for c in range(6):
                    nc.tensor.matmul(
                        psA[0:m, :], lhsT=w_t[:, c, wsl],
                        rhs=stg[:, c, :],
                        start=(c == 0), stop=(c == 5))
                nc.vector.tensor_scalar(
                    out=dst_fn(sl), in0=psA[0:m, :],
                    scalar1=bias_ap, scalar2=None, op0=ADD)

            def emit_proj_slice(ti, g):
                emit_proj_load(ti, g)
                for pi in range(len(projs[ti][2])):
                    emit_proj_part(ti, g, pi)

            def emit_vtrans(ts_):
                # all transposes from base partition 0 (base-64 transposes
                # give wrong data on HW) and same row-group (serial drains).
                for t in ts_:
                    sl = slice(t * 128, (t + 1) * 128)
                    psV = psum.tile([128, 512], BF16, tag="m5", bufs=2)
                    for h in range(3):
                        nc.tensor.transpose(psV[:, h * 64:(h + 1) * 64],
                                            VT3[0:64, h, sl], identB[0:64, 0:64])
                    nc.vector.tensor_copy(V_sb[:, t, :], psV[:, 0:NLOC])

            # ---- phase A helpers ----
            heads = [
                (QTa[0:64, :], KTa[0:64, :]),
                (QTa[64:128, :], KTa[64:128, :]),
                (QT2[0:64, :], KT2[0:64, :]),
            ]
            RZs = {}
            pv_state = {}

            def emit_N(h, sq):
                # natural scores tile -> exp(+rowsum) -> normalize -> HBM
                QT_h, KT_h = heads[h]
                RZ = RZs[h]
                P_t = pp.tile([128, S], F32, tag="P")
                z2 = zp.tile([128, 2], F32, tag="z2")
                for half in range(2):
                    psN = psum.tile([128, 1024], F32, tag="sc", bufs=2)
                    for kg in range(2):
                        nc.tensor.matmul(
                            psN[:, kg * 512:(kg + 1) * 512],
                            lhsT=QT_h[:, sq * 128:(sq + 1) * 128],
                            rhs=KT_h[:, (half * 2 + kg) * 512:(half * 2 + kg + 1) * 512],
                            start=True, stop=True)
                    nc.scalar.activation(
                        out=P_t[:, half * 1024:(half + 1) * 1024],
                        in_=psN[:], func=EXP,
                        accum_out=z2[:, half:half + 1])
                zt = zp.tile([128, 1], F32, tag="zt")
                nc.vector.tensor_add(zt[:], z2[:, 0:1], z2[:, 1:2])
                nc.vector.reciprocal_approx_fast(RZ[:, sq:sq + 1], zt[:])
                nc.vector.tensor_scalar_mul(P_t[:], P_t[:], RZ[:, sq:sq + 1])
                nc.sync.dma_start(
                    scores_d[h, sq * 128:(sq + 1) * 128, :], P_t[:])

            # transposed scores: per-chunk exp feeds PV immediately. PV(c-1)
            # is emitted after QK-T(c)+exp(c) so the PE never waits on ACT.
            def emit_T(h, u):
                QT_h, KT_h = heads[h]
                half, sk = divmod(u, 16)
                psT = psum.tile([128, 1024], F32, tag="sc", bufs=2)
                for qg in range(2):
                    nc.tensor.matmul(
                        psT[:, qg * 512:(qg + 1) * 512],
                        lhsT=KT_h[:, sk * 128:(sk + 1) * 128],
                        rhs=QT_h[:, half * 1024 + qg * 512:half * 1024 + (qg + 1) * 512],
                        start=True, stop=True)
                ET_c = etp.tile([128, 1024], BF16, tag="ET", bufs=18)
                nc.scalar.activation(out=ET_c[:], in_=psT[:], func=EXP)
                if sk == 0:
                    ps_pv = psum.tile([128, 512], F32, tag="pvx", bufs=2,
                                      name=f"pv_{h}_{half}")
                    pv_state[(h, half)] = (ps_pv, {})
                psPV, chunks = pv_state[(h, half)]
                chunks[sk] = ET_c
                return h, half, sk, psPV, chunks

            def emit_PV(h, half, sk, psPV, chunks):
                RZ = RZs[h]
                ET_c = chunks.pop(sk)
                for tq in range(8):
                    nc.tensor.matmul(
                        psPV[:, tq * 64:(tq + 1) * 64],
                        lhsT=ET_c[:, tq * 128:(tq + 1) * 128],
                        rhs=V_sb[:, sk, h * 64:(h + 1) * 64],
                        start=(sk == 0 and tq == 0),
                        stop=(sk == 15 and tq == 7))
                if sk == 15:
                    for tq in range(8):
                        sq = half * 8 + tq
                        nc.vector.tensor_scalar_mul(
                            ctxn[:, sq, h * 64:(h + 1) * 64],
                            psPV[:, tq * 64:(tq + 1) * 64],
                            RZ[:, sq:sq + 1])
                    return half
                return None

            def emit_out(half):
                # ctx^T transposes + output projection for one s_q half
                for sq in range(half * 8, half * 8 + 8):
                    sl = slice(sq * 128, (sq + 1) * 128)
                    psX = psum.tile([128, 512], BF16, tag="m5", bufs=2)
                    for h in range(3):
                        nc.tensor.transpose(
                            psX[0:64, h * 128:(h + 1) * 128],
                            ctxn[:, sq, h * 64:(h + 1) * 64], identB[:])
                    nc.vector.tensor_copy(ctxT[:, :, sl], psX[0:64, 0:384])
                for j in range(6):
                    oT = op.tile([128, 1024], F32, tag="oT")
                    jsl = slice(j * 128, (j + 1) * 128)
                    for g in range(2):
                        gsl = slice(half * 1024 + g * 512,
                                    half * 1024 + (g + 1) * 512)
                        psO = psum.tile([128, 512], F32, tag="m5", bufs=2)
                        for h in range(3):
                            nc.tensor.matmul(psO[:], lhsT=wot_t[:, h, jsl],
                                             rhs=ctxT[:, h, gsl],
                                             start=(h == 0), stop=(h == 2))
                        nc.vector.tensor_copy(oT[:, g * 512:(g + 1) * 512], psO[:])
                    nc.sync.dma_start(
                        outt_d[jsl, half * 1024:(half + 1) * 1024], oT[:])

            # ---- emission schedule ----
            # Preload the exp table set during the ramp with a dummy exp,
            # and warm up the PE clock (HAM) with ~3.5us of dummy matmuls so
            # the first projection matmuls run at 2.4 GHz.
            dummy = consts.tile([128, 1], F32)
            nc.vector.memset(dummy[:], 0.0)
            nc.scalar.activation(out=dummy[:], in_=dummy[:], func=EXP)
            warm_ps = psum.tile([128, 512], F32, tag="m5", bufs=2, name="warm_ps")
            for _ in range(34):
                nc.tensor.matmul(warm_ps[:, 0:128], lhsT=identB[:],
                                 rhs=identB[:], start=True, stop=True)
            warm_out = consts.tile([128, 128], F32)
            nc.vector.tensor_copy(warm_out[:], warm_ps[:, 0:128])

            RZs[0] = zp.tile([128, 16], F32, tag="RZ", name="RZ0")
            RZs[1] = zp.tile([128, 16], F32, tag="RZ", name="RZ1")
            RZs[2] = zp.tile([128, 16], F32, tag="RZ", name="RZ2")

            # Ramp: head0's half-0 T units only need q slices 0-1 and the k
            # slice covering their s_k block, so ACT starts after ~3 loads.
            # Their PV waits for V (deferred; ET chunks stay pinned).
            emit_proj_load(0, 0)
            load_w(wq_t, wqt_d)
            emit_proj_load(0, 1)
            emit_proj_load(1, 0)
            load_w(wk_t, wkt_d)
            emit_proj_part(0, 0, 0)
            emit_proj_part(0, 0, 1)
            emit_proj_part(0, 1, 0)
            emit_proj_part(0, 1, 1)
            emit_proj_part(1, 0, 0)
            emit_proj_part(1, 0, 1)
            deferred = []
            for blk in range(4):
                if blk < 3:
                    emit_proj_load(1, blk + 1)
                    emit_proj_part(1, blk + 1, 0)
                    emit_proj_part(1, blk + 1, 1)
                for sk in range(4 * blk, 4 * blk + 4):
                    deferred.append(emit_T(0, sk))

            # misc units woven into the early steady slots: remaining q
            # slices, head-2 row projections, all of V (+ transposes), and
            # the deferred head0-half0 PV drains right after each vt.
            units = []
            for g in (2, 3):
                units.append(("load", 0, g))
                units.append(("part", 0, g, 0))
                units.append(("part", 0, g, 1))
            units.append(("wv",))
            for g in range(4):             # v slices, transposes right behind
                units.append(("load", 2, g))
                for pi in range(3):
                    units.append(("part", 2, g, pi))
                units.append(("vt", 4 * g))
                units.append(("vt", 4 * g + 2))
                units.append(("pv", 4 * g))
                units.append(("pv", 4 * g + 2))
            units.append(("wo",))

            def pop_units(n):
                for _ in range(n):
                    if not units:
                        return
                    u = units.pop(0)
                    if u[0] == "load":
                        emit_proj_load(u[1], u[2])
                    elif u[0] == "part":
                        emit_proj_part(u[1], u[2], u[3])
                    elif u[0] == "vt":
                        emit_vtrans(range(u[1], u[1] + 2))
                    elif u[0] == "wv":
                        load_w(wv_t, wvt_d)
                    elif u[0] == "wo":
                        nc.gpsimd.dma_start(
                            out=wot_t,
                            in_=wot_d[:].rearrange("(h p) n -> p h n", p=64))
                    else:
                        emit_PV(*deferred[u[1]])
                        emit_PV(*deferred[u[1] + 1])

            # Global software pipeline over 48 slots: one N unit per slot
            # (DMA-heavy), remaining T units paced evenly from T_START
            # (ACT-heavy, no DMA). Keeps DMA and ACT both ~busy throughout.
            N_stream = [(h, sq) for h in range(3) for sq in range(16)]
            T_stream = [(0, u) for u in range(16, 32)]
            T_stream += [(h, u) for h in (1, 2) for u in range(32)]
            T_START = 7
            n_slots = len(N_stream)
            t_done = 0
            pending = None
            for slot in range(n_slots):
                emit_N(*N_stream[slot])
                pop_units(3)
                if slot >= T_START:
                    t_target = min(
                        len(T_stream),
                        (slot - T_START + 1) * len(T_stream)
                        // (n_slots - T_START),
                    )
                    while t_done < t_target:
                        nxt = emit_T(*T_stream[t_done])
                        t_done += 1
                        if pending is not None:
                            done = emit_PV(*pending)
                            if pending[0] == 2 and done is not None:
                                emit_out(done)
                        pending = nxt
            done = emit_PV(*pending)
            if done is not None:
                emit_out(done)

    nc.finalize()
    return nc


def _get_nc():
    if "nc" not in _CACHE:
        _CACHE["nc"] = _build_nc()
    return _CACHE["nc"]


def build_in_maps(inputs):
    q, k, v = inputs["q"], inputs["k"], inputs["v"]
    Wq, bq = inputs["Wq"], inputs["bq"]
    Wk, bk = inputs["Wk"], inputs["bk"]
    Wv, bv = inputs["Wv"], inputs["bv"]
    Wo = inputs["Wo"]
    f = np.float32
    q, k, v = (np.asarray(x, f) for x in (q, k, v))
    Wq, bq, Wk, bk, Wv, bv, Wo = (
        np.asarray(x, f) for x in (Wq, bq, Wk, bk, Wv, bv, Wo))

    import ml_dtypes
    bf16 = np.dtype(ml_dtypes.bfloat16)
    scale = f(1.0 / np.sqrt(DH))
    qts = [np.ascontiguousarray(q[b].T) for b in range(B)]
    kts = [np.ascontiguousarray(k[b].T) for b in range(B)]
    vts = [np.ascontiguousarray(v[b].T.astype(bf16)) for b in range(B)]

    in_maps = []
    for c in range(N_CORES):
        b, g = c // 4, c % 4
        sl = slice(g * NLOC, (g + 1) * NLOC)
        biasAB = np.stack(
            [bq[sl] * scale, bk[sl], bv[sl]], axis=1).astype(f)  # (192, 3)
        biasV01 = np.ascontiguousarray(
            bv[sl][0:128].reshape(2, 64).T.astype(f))  # (64, 2)
        in_maps.append({
            "qt": qts[b], "kt": kts[b], "vt": vts[b],
            "wqt": np.ascontiguousarray((Wq[sl] * scale).T),
            "wkt": np.ascontiguousarray(Wk[sl].T),
            "wvt": np.ascontiguousarray(Wv[sl].T.astype(bf16)),
            "wot": np.ascontiguousarray(Wo[:, sl].T),
            "biasA": np.ascontiguousarray(biasAB[0:128]),
            "biasB": np.ascontiguousarray(biasAB[128:NLOC]),
            "biasV": biasV01,
        })

    return in_maps


def kernel(q, k, v, Wq, bq, Wk, bk, Wv, bv, Wo, bo):
    global LAST_RESULT
    from concourse.bass_utils import run_bass_kernel_spmd

    f = np.float32
    bo = np.asarray(bo, f)
    in_maps = build_in_maps(dict(q=q, k=k, v=v, Wq=Wq, bq=bq, Wk=Wk, bk=bk,
                                 Wv=Wv, bv=bv, Wo=Wo, bo=bo))
    nc = _get_nc()
    LAST_RESULT = run_bass_kernel_spmd(nc, in_maps, list(range(N_CORES)))
    results = LAST_RESULT.results

    scores = np.empty((B, H, S, S), f)
    out_acc = np.zeros((B, D, S), f)
    for c, r in enumerate(results):
        b, g = c // 4, c % 4
        scores[b, g * HPC:(g + 1) * HPC] = r["scores"]
        out_acc[b] += r["outt"]
    out = np.ascontiguousarray(out_acc.transpose(0, 2, 1)) + bo
    return out, scores
